# revision 2
# baseline (speedup 1.0000x reference)
"""Trainium2 Bass kernel for nn_AttentionPromptExtrapolation.

Reference computation (B,N,P,D,K = 32,512,25,128,64):
    keep[n,p] = (n not in s_mti) and (p != 24)            # {0,1}, same for all b
    su = sigmoid(patches @ u.T);  su *= (su>0.5) * keep
    sm = sigmoid(patches @ m.T);  sm *= (sm>0.5) * (1-keep)
    out = patches + su @ u + sm @ m

Kernel formulation (halves the matmul work of the reference):
    C = concat(u, m)            [2K=128, D=128]
    z = x @ C.T                 [rows, 128]   (rows = flattened b,n,p)
    z += bias                   bias = -BIG where the (u|m) half is masked
                                (rank-2: outer(bcols, krows) via a K=2 matmul)
    st = (z > 0) * sigmoid(z)   (sigmoid(z)>0.5 <=> z>0; bias kills masked cols)
    out = x + st @ C

Per-core layout: data-parallel over B, 4 batches per core. Rows stream in
blocks of 512 (SBUF tile [128 part, 512 free], 2KB/partition contiguous DMA:
partition p holds rows 4p+s of the block, s=0..3). Scores are computed
transposed ([2K part, rows free]) so the second matmul gets its natural
stationary layout.
"""

import numpy as np
import ml_dtypes

import concourse.bass as bass
import concourse.bacc as bacc
import concourse.tile as tile
from concourse import mybir
from concourse.alu_op_type import AluOpType

B, N, P, D, K = 32, 512, 25, 128, 64
K2 = 2 * K              # 128
NCORES = 8
BPC = B // NCORES       # batches per core = 4
NP = N * P              # rows per batch = 12800
SUB = 4                 # rows per partition per block
BLK = 128 * SUB         # rows per block = 512
NBLK = NP // BLK        # blocks per batch = 25
T_MTI = 24
BIG = 1e30

F32 = mybir.dt.float32
F16 = mybir.dt.float16
BF16 = mybir.dt.bfloat16


def build_nc(n_blocks=BPC * NBLK):
    """Build the single-core bass program. n_blocks can be reduced for sim tests."""
    rows = n_blocks * BLK
    nc = bacc.Bacc(None, target_bir_lowering=False)

    x_d = nc.dram_tensor("x", [rows, D], F32, kind="ExternalInput")
    ct_d = nc.dram_tensor("ct", [D, K2], F32, kind="ExternalInput")       # C.T
    cfp_d = nc.dram_tensor("cfp", [K2, D], F16, kind="ExternalInput")     # C fp16
    bcols_d = nc.dram_tensor("bcols", [2, K2], BF16, kind="ExternalInput")
    krows_d = nc.dram_tensor("krows", [2, NP], BF16, kind="ExternalInput")
    ident_d = nc.dram_tensor("ident", [128, 128], F32, kind="ExternalInput")
    out_d = nc.dram_tensor("out", [rows, D], F32, kind="ExternalOutput")

    # [blk, 128 part, (s d)]: partition p of block t holds rows 512t+4p+s
    x_r = x_d[:, :].rearrange("(blk p s) d -> blk p (s d)", p=128, s=SUB)
    out_r = out_d[:, :].rearrange("(blk p s) d -> blk p (s d)", p=128, s=SUB)

    with tile.TileContext(nc) as tc:
        with (
            tc.tile_pool(name="consts", bufs=1) as consts,
            tc.tile_pool(name="xp", bufs=4) as xp,
            tc.tile_pool(name="xtp", bufs=3) as xtp,
            tc.tile_pool(name="sgp", bufs=3) as sgp,
            tc.tile_pool(name="stp", bufs=3) as stp,
            tc.tile_pool(name="op", bufs=4) as op,
            tc.tile_pool(name="ps_xt", bufs=2, space="PSUM") as ps_xt,
            tc.tile_pool(name="ps_z", bufs=2, space="PSUM") as ps_z,
            tc.tile_pool(name="ps_y", bufs=2, space="PSUM") as ps_y,
        ):
            ct_sb = consts.tile([D, K2], F32)
            nc.sync.dma_start(ct_sb, ct_d[:, :])
            cfp_sb = consts.tile([K2, D], F16)
            nc.sync.dma_start(cfp_sb, cfp_d[:, :])
            bcols_sb = consts.tile([2, K2], BF16)
            nc.sync.dma_start(bcols_sb, bcols_d[:, :])
            krows_sb = consts.tile([2, NP], BF16)
            nc.sync.dma_start(krows_sb, krows_d[:, :])
            ident_sb = consts.tile([128, 128], F32)
            nc.sync.dma_start(ident_sb, ident_d[:, :])

            for blk in range(n_blocks):
                t = blk % NBLK
                x_sb = xp.tile([128, BLK], F32)
                nc.sync.dma_start(x_sb, x_r[blk])

                # PE transpose of the 4 [128,128] sub-tiles -> Xt [d, rows]
                xt_ps = ps_xt.tile([128, BLK], F32)
                for j in range(SUB):
                    nc.tensor.transpose(
                        xt_ps[:, j * 128:(j + 1) * 128],
                        x_sb[:, j * 128:(j + 1) * 128],
                        ident_sb,
                    )
                xt_sb = xtp.tile([128, BLK], F32)
                nc.scalar.copy(xt_sb, xt_ps)

                # z.T [2K, rows] = C @ x.T  (+ mask bias via K=2 matmul)
                z_ps = ps_z.tile([128, BLK], F32)
                nc.tensor.matmul(z_ps, lhsT=ct_sb, rhs=xt_sb, start=True, stop=False)
                nc.tensor.matmul(
                    z_ps,
                    lhsT=bcols_sb,
                    rhs=krows_sb[:, t * BLK:(t + 1) * BLK],
                    start=False,
                    stop=True,
                )

                sig_sb = sgp.tile([128, BLK], F16)
                nc.scalar.activation(
                    sig_sb, z_ps, mybir.ActivationFunctionType.Sigmoid
                )
                # st = (z > 0) * sigmoid(z)
                st_sb = stp.tile([128, BLK], F16)
                nc.vector.scalar_tensor_tensor(
                    out=st_sb,
                    in0=z_ps,
                    scalar=0.0,
                    in1=sig_sb,
                    op0=AluOpType.is_gt,
                    op1=AluOpType.mult,
                )

                # y [rows, D] = st.T @ C, one matmul per 128-row sub-tile
                y_ps = ps_y.tile([128, BLK], F32)
                for j in range(SUB):
                    nc.tensor.matmul(
                        y_ps[:, j * 128:(j + 1) * 128],
                        lhsT=st_sb[:, j * 128:(j + 1) * 128],
                        rhs=cfp_sb,
                        start=True,
                        stop=True,
                    )

                o_sb = op.tile([128, BLK], F32)
                nc.vector.tensor_tensor(out=o_sb, in0=x_sb, in1=y_ps, op=AluOpType.add)
                nc.sync.dma_start(out_r[blk], o_sb)

    nc.compile()
    return nc


def host_inputs(patches, u_prompt, m_prompt, s_mti):
    """Build the per-core input maps (host-side prep of tables/constants)."""
    patches = np.asarray(patches, dtype=np.float32)
    u = np.asarray(u_prompt, dtype=np.float32)
    m = np.asarray(m_prompt, dtype=np.float32)
    s_mti = np.asarray(s_mti)

    C = np.concatenate([u, m], axis=0)                     # [128, 128]
    ct = np.ascontiguousarray(C.T)                         # [D, 2K] f32
    cfp = C.astype(ml_dtypes.float16 if hasattr(ml_dtypes, "float16") else np.float16)
    cfp = np.ascontiguousarray(cfp.astype(np.float16))

    n_mask = np.ones(N, np.float32)
    n_mask[s_mti] = 0.0
    t_mask = np.ones(P, np.float32)
    t_mask[T_MTI] = 0.0
    keep = (n_mask[:, None] * t_mask[None, :]).reshape(-1)  # [NP]
    # permute to the (t, j, p) free order of the score tiles:
    # position t*512 + j*128 + p  <->  row 512t + 4p + j
    kperm = keep.reshape(NBLK, 128, SUB).transpose(0, 2, 1).reshape(-1)
    krows = np.stack([-BIG * (1.0 - kperm), -BIG * kperm]).astype(ml_dtypes.bfloat16)

    bcols = np.zeros((2, K2), np.float32)
    bcols[0, :K] = 1.0
    bcols[1, K:] = 1.0
    bcols = bcols.astype(ml_dtypes.bfloat16)

    ident = np.eye(128, dtype=np.float32)

    x_flat = patches.reshape(B, NP * D // D, D).reshape(B, -1, D)  # [B, NP, D]
    in_maps = []
    for c in range(NCORES):
        xs = np.ascontiguousarray(
            x_flat[c * BPC:(c + 1) * BPC].reshape(BPC * NP, D)
        )
        in_maps.append({
            "x": xs,
            "ct": ct,
            "cfp": cfp,
            "bcols": bcols,
            "krows": krows,
            "ident": ident,
        })
    return in_maps


_NC_CACHE = {}


def kernel(patches, u_prompt, m_prompt, s_mti, s_uti=None, trace=False, **kw):
    from concourse.bass_utils import run_bass_kernel_spmd

    in_maps = host_inputs(patches, u_prompt, m_prompt, s_mti)

    if "nc" not in _NC_CACHE:
        _NC_CACHE["nc"] = build_nc()
    nc = _NC_CACHE["nc"]

    res = run_bass_kernel_spmd(nc, in_maps, list(range(NCORES)), trace=trace)
    outs = [res.results[c]["out"] for c in range(NCORES)]
    out = np.concatenate(
        [o.reshape(BPC, N, P, D) for o in outs], axis=0
    ).astype(np.float32)
    if trace:
        kernel.last_results = res
    return out


# revision 4
# speedup vs baseline: 1.4577x; 1.4577x over previous
"""Trainium2 Bass kernel for nn_AttentionPromptExtrapolation.

Reference computation (B,N,P,D,K = 32,512,25,128,64):
    keep[n,p] = (n not in s_mti) and (p != 24)            # {0,1}, same for all b
    su = sigmoid(patches @ u.T);  su *= (su>0.5) * keep
    sm = sigmoid(patches @ m.T);  sm *= (sm>0.5) * (1-keep)
    out = patches + su @ u + sm @ m

Kernel formulation (halves the matmul work of the reference):
    C = concat(u, m)            [2K=128, D=128]
    z = x @ C.T                 [rows, 128]   (rows = flattened b,n,p)
    z += bias                   bias = -BIG where the (u|m) half is masked
                                (rank-2: outer(bcols, krows) via a K=2 matmul)
    st = (z > 0) * sigmoid(z)   (sigmoid(z)>0.5 <=> z>0; bias kills masked cols)
    out = x + st @ C

Per-core layout: data-parallel over B, 4 batches per core. Rows stream in
blocks of 512 (SBUF tile [128 part, 512 free], 2KB/partition contiguous DMA:
partition p holds rows 4p+s of the block, s=0..3). Scores are computed
transposed ([2K part, rows free]) so the second matmul gets its natural
stationary layout.
"""

import numpy as np
import ml_dtypes

import concourse.bass as bass
import concourse.bacc as bacc
import concourse.tile as tile
from concourse import mybir
from concourse.alu_op_type import AluOpType

B, N, P, D, K = 32, 512, 25, 128, 64
K2 = 2 * K              # 128
NCORES = 8
BPC = B // NCORES       # batches per core = 4
NP = N * P              # rows per batch = 12800
SUB = 4                 # rows per partition per block
BLK = 128 * SUB         # rows per block = 512
NBLK = NP // BLK        # blocks per batch = 25
T_MTI = 24
BIG = 1e30

F32 = mybir.dt.float32
F16 = mybir.dt.float16
BF16 = mybir.dt.bfloat16


MB = 5                   # sub-blocks (512 rows) per megablock DMA
MBROWS = MB * BLK        # 2560 rows = 1.25 MB per DMA
SUBROWS = MB * SUB       # 20 rows per partition per megablock


def build_nc(n_blocks=BPC * NBLK):
    """Build the single-core bass program. n_blocks can be reduced for sim
    tests (must be a multiple of MB=5)."""
    assert n_blocks % MB == 0
    n_mb = n_blocks // MB
    rows = n_blocks * BLK
    nc = bacc.Bacc(None, target_bir_lowering=False)

    x_d = nc.dram_tensor("x", [rows, D], F32, kind="ExternalInput")
    ct_d = nc.dram_tensor("ct", [D, K2], F32, kind="ExternalInput")       # C.T
    cfp_d = nc.dram_tensor("cfp", [K2, D], F16, kind="ExternalInput")     # C fp16
    bcols_d = nc.dram_tensor("bcols", [2, K2], BF16, kind="ExternalInput")
    krows_d = nc.dram_tensor("krows", [2, NP], BF16, kind="ExternalInput")
    ident_d = nc.dram_tensor("ident", [128, 128], F32, kind="ExternalInput")
    out_d = nc.dram_tensor("out", [rows, D], F32, kind="ExternalOutput")

    # [mb, 128 part, (s d)]: partition p of megablock mb holds rows
    # mb*2560 + 20p + s (s = 0..19), 10 KB contiguous per partition.
    x_r = x_d[:, :].rearrange("(mb p s) d -> mb p (s d)", p=128, s=SUBROWS)
    out_r = out_d[:, :].rearrange("(mb p s) d -> mb p (s d)", p=128, s=SUBROWS)

    with tile.TileContext(nc) as tc:
        with (
            tc.tile_pool(name="consts", bufs=1) as consts,
            tc.tile_pool(name="xp", bufs=3) as xp,
            tc.tile_pool(name="xtp", bufs=3) as xtp,
            tc.tile_pool(name="sgp", bufs=3) as sgp,
            tc.tile_pool(name="stp", bufs=3) as stp,
            tc.tile_pool(name="op", bufs=2) as op,
            tc.tile_pool(name="ps_xt", bufs=2, space="PSUM") as ps_xt,
            tc.tile_pool(name="ps_z", bufs=2, space="PSUM") as ps_z,
            tc.tile_pool(name="ps_y", bufs=2, space="PSUM") as ps_y,
        ):
            ct_sb = consts.tile([D, K2], F32)
            nc.sync.dma_start(ct_sb, ct_d[:, :])
            cfp_sb = consts.tile([K2, D], F16)
            nc.sync.dma_start(cfp_sb, cfp_d[:, :])
            bcols_sb = consts.tile([2, K2], BF16)
            nc.sync.dma_start(bcols_sb, bcols_d[:, :])
            krows_sb = consts.tile([2, NP], BF16)
            nc.sync.dma_start(krows_sb, krows_d[:, :])
            ident_sb = consts.tile([128, 128], F32)
            nc.sync.dma_start(ident_sb, ident_d[:, :])

            x_mb = o_mb = None
            pend = None  # (st_sb, x_mb, o_mb, sub) of the previous sub-block

            def flush(pend):
                st_sb, px_mb, po_mb, psub, pmb = pend
                # y [rows, D] = st.T @ C, one matmul per 128-row sub-tile
                y_ps = ps_y.tile([128, BLK], F32)
                for j in range(SUB):
                    nc.tensor.matmul(
                        y_ps[:, j * 128:(j + 1) * 128],
                        lhsT=st_sb[:, j * 128:(j + 1) * 128],
                        rhs=cfp_sb,
                        start=True,
                        stop=True,
                    )
                nc.vector.tensor_tensor(
                    out=po_mb[:, psub * BLK:(psub + 1) * BLK],
                    in0=px_mb[:, psub * BLK:(psub + 1) * BLK],
                    in1=y_ps,
                    op=AluOpType.add,
                )
                if psub == MB - 1:
                    nc.sync.dma_start(out_r[pmb], po_mb)

            for i in range(n_blocks):
                mb, sub = divmod(i, MB)
                if sub == 0:
                    x_mb = xp.tile([128, MBROWS], F32)
                    nc.sync.dma_start(x_mb, x_r[mb])
                    o_mb = op.tile([128, MBROWS], F32)

                # PE transpose of the 4 [128,128] sub-tiles -> Xt [d, rows]
                xt_ps = ps_xt.tile([128, BLK], F32)
                for j in range(SUB):
                    c = sub * SUB + j
                    nc.tensor.transpose(
                        xt_ps[:, j * 128:(j + 1) * 128],
                        x_mb[:, c * 128:(c + 1) * 128],
                        ident_sb,
                    )
                xt_sb = xtp.tile([128, BLK], F32)
                nc.scalar.copy(xt_sb, xt_ps)

                # z.T [2K, rows] = C @ x.T  (+ mask bias via K=2 matmul)
                z_ps = ps_z.tile([128, BLK], F32)
                nc.tensor.matmul(z_ps, lhsT=ct_sb, rhs=xt_sb, start=True, stop=False)
                t = i % NBLK
                nc.tensor.matmul(
                    z_ps,
                    lhsT=bcols_sb,
                    rhs=krows_sb[:, t * BLK:(t + 1) * BLK],
                    start=False,
                    stop=True,
                )

                sig_sb = sgp.tile([128, BLK], F16)
                nc.scalar.activation(
                    sig_sb, z_ps, mybir.ActivationFunctionType.Sigmoid
                )
                # st = (z > 0) * sigmoid(z)
                st_sb = stp.tile([128, BLK], F16)
                nc.vector.scalar_tensor_tensor(
                    out=st_sb,
                    in0=z_ps,
                    scalar=0.0,
                    in1=sig_sb,
                    op0=AluOpType.is_gt,
                    op1=AluOpType.mult,
                )

                # run the second-matmul/add/store stage one sub-block behind
                # so PE never stalls waiting on sigmoid/STT of the current one
                if pend is not None:
                    flush(pend)
                pend = (st_sb, x_mb, o_mb, sub, mb)

            flush(pend)

    nc.compile()
    return nc


def host_inputs(patches, u_prompt, m_prompt, s_mti):
    """Build the per-core input maps (host-side prep of tables/constants)."""
    patches = np.asarray(patches, dtype=np.float32)
    u = np.asarray(u_prompt, dtype=np.float32)
    m = np.asarray(m_prompt, dtype=np.float32)
    s_mti = np.asarray(s_mti)

    C = np.concatenate([u, m], axis=0)                     # [128, 128]
    ct = np.ascontiguousarray(C.T)                         # [D, 2K] f32
    cfp = C.astype(ml_dtypes.float16 if hasattr(ml_dtypes, "float16") else np.float16)
    cfp = np.ascontiguousarray(cfp.astype(np.float16))

    n_mask = np.ones(N, np.float32)
    n_mask[s_mti] = 0.0
    t_mask = np.ones(P, np.float32)
    t_mask[T_MTI] = 0.0
    keep = (n_mask[:, None] * t_mask[None, :]).reshape(-1)  # [NP]
    # permute to the (megablock, c, p) free order of the score tiles:
    # position mb*2560 + c*128 + p  <->  row mb*2560 + 20p + c
    kperm = keep.reshape(NP // MBROWS, 128, SUBROWS).transpose(0, 2, 1).reshape(-1)
    krows = np.stack([-BIG * (1.0 - kperm), -BIG * kperm]).astype(ml_dtypes.bfloat16)

    bcols = np.zeros((2, K2), np.float32)
    bcols[0, :K] = 1.0
    bcols[1, K:] = 1.0
    bcols = bcols.astype(ml_dtypes.bfloat16)

    ident = np.eye(128, dtype=np.float32)

    x_flat = patches.reshape(B, NP * D // D, D).reshape(B, -1, D)  # [B, NP, D]
    in_maps = []
    for c in range(NCORES):
        xs = np.ascontiguousarray(
            x_flat[c * BPC:(c + 1) * BPC].reshape(BPC * NP, D)
        )
        in_maps.append({
            "x": xs,
            "ct": ct,
            "cfp": cfp,
            "bcols": bcols,
            "krows": krows,
            "ident": ident,
        })
    return in_maps


_NC_CACHE = {}


def kernel(patches, u_prompt, m_prompt, s_mti, s_uti=None, trace=False, **kw):
    from concourse.bass_utils import run_bass_kernel_spmd

    in_maps = host_inputs(patches, u_prompt, m_prompt, s_mti)

    if "nc" not in _NC_CACHE:
        _NC_CACHE["nc"] = build_nc()
    nc = _NC_CACHE["nc"]

    res = run_bass_kernel_spmd(nc, in_maps, list(range(NCORES)), trace=trace)
    outs = [res.results[c]["out"] for c in range(NCORES)]
    out = np.concatenate(
        [o.reshape(BPC, N, P, D) for o in outs], axis=0
    ).astype(np.float32)
    if trace:
        kernel.last_results = res
    return out


# revision 5
# speedup vs baseline: 1.7673x; 1.2124x over previous
"""Trainium2 Bass kernel for nn_AttentionPromptExtrapolation.

Reference computation (B,N,P,D,K = 32,512,25,128,64):
    keep[n,p] = (n not in s_mti) and (p != 24)            # {0,1}, same for all b
    su = sigmoid(patches @ u.T);  su *= (su>0.5) * keep
    sm = sigmoid(patches @ m.T);  sm *= (sm>0.5) * (1-keep)
    out = patches + su @ u + sm @ m

Kernel formulation (halves the matmul work of the reference):
    C = concat(u, m)            [2K=128, D=128]
    z = x @ C.T                 [rows, 128]   (rows = flattened b,n,p)
    z += bias                   bias = -BIG where the (u|m) half is masked
                                (rank-2: outer(bcols, krows) via a K=2 matmul)
    st = (z > 0) * sigmoid(z)   (sigmoid(z)>0.5 <=> z>0; bias kills masked cols)
    out = x + st @ C

Per-core: data-parallel over B, 4 batches per core. The host ships patches
TRANSPOSED ([D, rows] row-major) so the contraction dim D sits on SBUF
partitions with perfectly contiguous 10KB-per-partition DMA chunks — no
on-chip transposes at all. The whole pipeline runs in transposed space
(scores [2K, rows], contribution yT [D, rows], output [D, rows]); the host
un-transposes the result. All matmul stationaries are constants.
The second matmul / add / store stage runs one sub-block behind the score
stage so the PE never stalls waiting on sigmoid/STT.
"""

import numpy as np
import ml_dtypes

import concourse.bass as bass
import concourse.bacc as bacc
import concourse.tile as tile
from concourse import mybir
from concourse.alu_op_type import AluOpType

B, N, P, D, K = 32, 512, 25, 128, 64
K2 = 2 * K              # 128
NCORES = 8
BPC = B // NCORES       # batches per core = 4
NP = N * P              # rows per batch = 12800
BLK = 512               # rows per compute sub-block (one PSUM bank)
NBLK = NP // BLK        # sub-blocks per batch = 25
MB = 5                  # sub-blocks per megablock DMA
MBROWS = MB * BLK       # 2560 rows = 1.25 MB per DMA
T_MTI = 24
BIG = 1e30

F32 = mybir.dt.float32
F16 = mybir.dt.float16
BF16 = mybir.dt.bfloat16


def build_nc(n_blocks=BPC * NBLK):
    """Build the single-core bass program. n_blocks can be reduced for sim
    tests (must be a multiple of MB=5)."""
    assert n_blocks % MB == 0
    n_mb = n_blocks // MB
    rows = n_blocks * BLK
    nc = bacc.Bacc(None, target_bir_lowering=False)

    x_d = nc.dram_tensor("x", [D, rows], F32, kind="ExternalInput")       # x.T
    ct_d = nc.dram_tensor("ct", [D, K2], F32, kind="ExternalInput")       # C.T
    cfp_d = nc.dram_tensor("cfp", [K2, D], F16, kind="ExternalInput")     # C fp16
    bcols_d = nc.dram_tensor("bcols", [2, K2], BF16, kind="ExternalInput")
    krows_d = nc.dram_tensor("krows", [2, NP], BF16, kind="ExternalInput")
    out_d = nc.dram_tensor("out", [D, rows], F32, kind="ExternalOutput")  # out.T

    with tile.TileContext(nc) as tc:
        with (
            tc.tile_pool(name="consts", bufs=1) as consts,
            tc.tile_pool(name="xp", bufs=3) as xp,
            tc.tile_pool(name="sgp", bufs=3) as sgp,
            tc.tile_pool(name="stp", bufs=3) as stp,
            tc.tile_pool(name="op", bufs=2) as op,
            tc.tile_pool(name="ps_z", bufs=3, space="PSUM") as ps_z,
            tc.tile_pool(name="ps_y", bufs=3, space="PSUM") as ps_y,
        ):
            ct_sb = consts.tile([D, K2], F32)
            nc.sync.dma_start(ct_sb, ct_d[:, :])
            cfp_sb = consts.tile([K2, D], F16)
            nc.sync.dma_start(cfp_sb, cfp_d[:, :])
            bcols_sb = consts.tile([2, K2], BF16)
            nc.sync.dma_start(bcols_sb, bcols_d[:, :])
            krows_sb = consts.tile([2, NP], BF16)
            nc.sync.dma_start(krows_sb, krows_d[:, :])

            x_mb = o_mb = None
            pend = None  # previous sub-block's (st_sb, x_mb, o_mb, sub, mb)

            def flush(pend):
                st_sb, px_mb, po_mb, psub, pmb = pend
                # yT [D, rows] = C.T @ st — constant stationary, one matmul
                y_ps = ps_y.tile([128, BLK], F32)
                nc.tensor.matmul(y_ps, lhsT=cfp_sb, rhs=st_sb, start=True, stop=True)
                nc.vector.tensor_tensor(
                    out=po_mb[:, psub * BLK:(psub + 1) * BLK],
                    in0=px_mb[:, psub * BLK:(psub + 1) * BLK],
                    in1=y_ps,
                    op=AluOpType.add,
                )
                if psub == MB - 1:
                    nc.sync.dma_start(
                        out_d[:, pmb * MBROWS:(pmb + 1) * MBROWS], po_mb
                    )

            for i in range(n_blocks):
                mb, sub = divmod(i, MB)
                if sub == 0:
                    x_mb = xp.tile([128, MBROWS], F32)
                    nc.sync.dma_start(x_mb, x_d[:, mb * MBROWS:(mb + 1) * MBROWS])
                    o_mb = op.tile([128, MBROWS], F32)

                # z.T [2K, rows] = C @ x.T  (+ mask bias via K=2 matmul)
                z_ps = ps_z.tile([128, BLK], F32)
                nc.tensor.matmul(
                    z_ps,
                    lhsT=ct_sb,
                    rhs=x_mb[:, sub * BLK:(sub + 1) * BLK],
                    start=True,
                    stop=False,
                )
                t = i % NBLK
                nc.tensor.matmul(
                    z_ps,
                    lhsT=bcols_sb,
                    rhs=krows_sb[:, t * BLK:(t + 1) * BLK],
                    start=False,
                    stop=True,
                )

                sig_sb = sgp.tile([128, BLK], F16)
                nc.scalar.activation(
                    sig_sb, z_ps, mybir.ActivationFunctionType.Sigmoid
                )
                # st = (z > 0) * sigmoid(z)
                st_sb = stp.tile([128, BLK], F16)
                nc.vector.scalar_tensor_tensor(
                    out=st_sb,
                    in0=z_ps,
                    scalar=0.0,
                    in1=sig_sb,
                    op0=AluOpType.is_gt,
                    op1=AluOpType.mult,
                )

                # second matmul / add / store run one sub-block behind so the
                # PE never waits on the current sub-block's sigmoid/STT
                if pend is not None:
                    flush(pend)
                pend = (st_sb, x_mb, o_mb, sub, mb)

            flush(pend)

    nc.compile()
    return nc


def host_inputs(patches, u_prompt, m_prompt, s_mti):
    """Build the per-core input maps (host-side prep of tables/constants)."""
    patches = np.asarray(patches, dtype=np.float32)
    u = np.asarray(u_prompt, dtype=np.float32)
    m = np.asarray(m_prompt, dtype=np.float32)
    s_mti = np.asarray(s_mti)

    C = np.concatenate([u, m], axis=0)                     # [128, 128]
    ct = np.ascontiguousarray(C.T)                         # [D, 2K] f32
    cfp = np.ascontiguousarray(C.astype(np.float16))

    n_mask = np.ones(N, np.float32)
    n_mask[s_mti] = 0.0
    t_mask = np.ones(P, np.float32)
    t_mask[T_MTI] = 0.0
    keep = (n_mask[:, None] * t_mask[None, :]).reshape(-1)  # [NP]
    krows = np.stack([-BIG * (1.0 - keep), -BIG * keep]).astype(ml_dtypes.bfloat16)

    bcols = np.zeros((2, K2), np.float32)
    bcols[0, :K] = 1.0
    bcols[1, K:] = 1.0
    bcols = bcols.astype(ml_dtypes.bfloat16)

    x_flat = patches.reshape(B, NP, D)
    in_maps = []
    for c in range(NCORES):
        xs = np.ascontiguousarray(
            x_flat[c * BPC:(c + 1) * BPC].reshape(BPC * NP, D).T
        )
        in_maps.append({
            "x": xs,
            "ct": ct,
            "cfp": cfp,
            "bcols": bcols,
            "krows": krows,
        })
    return in_maps


_NC_CACHE = {}


def kernel(patches, u_prompt, m_prompt, s_mti, s_uti=None, trace=False, **kw):
    from concourse.bass_utils import run_bass_kernel_spmd

    in_maps = host_inputs(patches, u_prompt, m_prompt, s_mti)

    if "nc" not in _NC_CACHE:
        _NC_CACHE["nc"] = build_nc()
    nc = _NC_CACHE["nc"]

    res = run_bass_kernel_spmd(nc, in_maps, list(range(NCORES)), trace=trace)
    outs = [res.results[c]["out"] for c in range(NCORES)]   # each [D, BPC*NP]
    out = np.concatenate(
        [np.ascontiguousarray(o.T).reshape(BPC, N, P, D) for o in outs], axis=0
    ).astype(np.float32)
    if trace:
        kernel.last_results = res
    return out
